# revision 41
# baseline (speedup 1.0000x reference)
"""LoRA multi-head attention on 8 TRN2 NeuronCores.

Sharding: data-parallel over batch (B=8 -> 1 batch element per core),
weights replicated, no collectives.

Host side: LoRA is folded into the dense weights exactly
(W' = W + (alpha/r) * B @ A), the attention scale 1/8 is folded into
Wq (power of two => lossless), and bo is pre-replicated across
partitions. The device then runs a plain dense MHA in bf16 with fp32
PSUM accumulation.

Device pipeline per core:
  qT/kT per dout-tile dt: (WT.T @ xT); v natural per token-tile with a
  ones column ([v_h | 1]) so PV also yields softmax denominators.
  Attention per dt (2 heads), per query-half nh, per key-tile mt:
    S^T pair via 64-row PE tiling (both heads concurrent) -> exp on
    ACT (psum -> bf16 sbuf) -> PV (M=65).  The next dout-tile's q/k
    projection matmuls are woven between S and PV so the PE never
    waits on the ACT exp latency.
  Normalization: reciprocal of the denom row (bf16) broadcast via a
  K=1 bf16 matmul; DVE multiply writes attnT directly (head 0) or via
  a staging tile + SBUF DMA (head 1).
  Output projection: kt=0..6 partial sums woven into the dt=7
  attention loop (+bo), tail adds only the kt=7 term.
"""

import sys

if "/opt/trn_rl_repo" not in sys.path:
    sys.path.insert(0, "/opt/trn_rl_repo")

import numpy as np
import ml_dtypes

BF16 = ml_dtypes.bfloat16

N = 1024  # tokens
D = 1024  # model dim
H = 16    # heads
HD = 64   # head dim
R = 16    # lora rank
P = 128   # partitions
F = 512   # psum free-dim tile
NCORES = 8
SCALING = 1.0 / 16.0  # lora alpha/rank
SCALE = HD ** -0.5

_CACHE = {}


def _build():
    import concourse.bacc as bacc
    import concourse.mybir as mybir
    import concourse.tile as tile

    f32 = mybir.dt.float32
    bf16 = mybir.dt.bfloat16
    Exp = mybir.ActivationFunctionType.Exp

    nc = bacc.Bacc("TRN2", target_bir_lowering=False, debug=False)

    xT_e = nc.declare_dram_parameter("xT", [D, N], bf16, isOutput=False)
    wT_e = {
        nm: nc.declare_dram_parameter(nm, [D, D], bf16, isOutput=False)
        for nm in ("WqT", "WkT", "WvT", "WoT")
    }
    bo_e = nc.declare_dram_parameter("boR", [P, D], bf16, isOutput=False)
    out_e = nc.declare_dram_parameter("out", [N, D], bf16, isOutput=True)

    with tile.TileContext(nc) as tc:
        with (
            tc.tile_pool(name="wpool", bufs=1) as wpool,
            tc.tile_pool(name="stage", bufs=2) as stage,
            tc.tile_pool(name="ps", bufs=1, space="PSUM") as ps,
        ):
            # ---- DMA loads: x first (5 queues), then weights ----
            dq = [nc.sync, nc.scalar, nc.gpsimd]
            qi = 0
            T = {}

            def load_big(nm, ext, qs3=False):
                nonlocal qi
                T[nm] = []
                for t in range(8):
                    tt = wpool.tile([P, D], bf16, tag=f"T_{nm}_{t}",
                                    name=f"T_{nm}_{t}")
                    eng = dq[qi % 3] if qs3 else [nc.sync, nc.gpsimd][qi % 2]
                    eng.dma_start(out=tt[:], in_=ext[t * P:(t + 1) * P, :])
                    qi += 1
                    T[nm].append(tt)

            # x spread over 3 queues (gates everything); Wq/Wk dt=0/1
            # columns sliced so dt=0 projections start at ~9us; Wv next
            # (v-projection); the rest as big region loads behind compute.
            # Steady-state queue roles: sync=ast/out, scalar=exp + dn0,
            # gpsimd=partition_broadcast.
            wq2 = [nc.sync, nc.gpsimd]

            def load_qk_cols(dts, ts=range(8)):
                nonlocal qi
                c0, c1 = dts[0] * P, dts[-1] * P + P
                for nm in ("Wq", "Wk"):
                    if nm not in T:
                        T[nm] = [wpool.tile([P, D], bf16,
                                            tag=f"T_{nm}_{t}",
                                            name=f"T_{nm}_{t}")
                                 for t in range(8)]
                    for t in ts:
                        wq2[qi % 2].dma_start(
                            out=T[nm][t][:, c0:c1],
                            in_=wT_e[nm + "T"][t * P:(t + 1) * P, c0:c1])
                        qi += 1

            # interleave x tiles with the dt=0 W column slices so the
            # first q-projection accumulation chain starts immediately
            T["x"] = [wpool.tile([P, D], bf16, tag=f"T_x_{t}",
                                 name=f"T_x_{t}") for t in range(8)]
            for t in range(8):
                dq[t % 3].dma_start(out=T["x"][t][:],
                                    in_=xT_e[t * P:(t + 1) * P, :])
                load_qk_cols([0], ts=[t])
            load_big("Wv", wT_e["WvT"])
            load_qk_cols([1])
            load_qk_cols([2, 3, 4, 5, 6, 7])
            load_big("Wo", wT_e["WoT"])
            bo_sb = wpool.tile([P, D], bf16, tag="bo")
            nc.gpsimd.dma_start(out=bo_sb[:], in_=bo_e[:, :])

            ones1 = wpool.tile([1, HD], bf16, tag="ones1")
            nc.vector.memset(ones1[:], 1.0)
            warm = wpool.tile([P, F], bf16, tag="warm")
            nc.vector.memset(warm[:], 0.0)

            # ---- ACT exp-table preload + PE warm-up (no DMA deps) ----
            wexp = stage.tile([P, F], bf16, tag="wexp", bufs=1)
            wps = ps.tile([P, F], f32, tag="wk", bufs=2)
            nc.tensor.matmul(wps[:], warm[:, 0:P], warm[:], start=True,
                             stop=True)
            nc.scalar.activation(wexp[:], wps[:], Exp)
            for _ in range(5):
                nc.tensor.matmul(wps[:], warm[:, 0:P], warm[:],
                                 start=True, stop=True)

            # ---- q/k projection generator for dout tile dt ----
            qks = {}

            def proj_gen(dt):
                qk = {}
                for nm, wnm in (("q", "Wq"), ("k", "Wk")):
                    dst = wpool.tile([P, D], bf16, tag=f"{nm}T",
                                     bufs=3, name=f"{nm}T_{dt}")
                    qk[nm] = dst
                    for nh in range(2):
                        ns = slice(nh * F, (nh + 1) * F)
                        pq = ps.tile([P, F], f32, tag="wk", bufs=2)
                        for kt in range(8):
                            nc.tensor.matmul(
                                pq[:], T[wnm][kt][:, dt * P:(dt + 1) * P],
                                T["x"][kt][:, ns],
                                start=(kt == 0), stop=(kt == 7))
                            yield
                        nc.vector.tensor_copy(dst[:, ns], pq[:])
                        yield
                qks[dt] = qk

            # ---- output-projection partials (kt=0..6) for dt=7 weave ----
            attnT = [wpool.tile([P, D], bf16, tag=f"attnT_{t}",
                                name=f"attnT_{t}") for t in range(8)]
            partials = [wpool.tile([P, F], bf16, tag=f"part_{t}",
                                   name=f"part_{t}") for t in range(16)]

            def out_chunk(kts, first):
                # partial[nt,dh] (+)= sum_{kt in kts} attnT[kt]^T @ Wo[kt]
                for nt in range(8):
                    for dh in range(2):
                        ds = slice(dh * F, (dh + 1) * F)
                        pf = ps.tile([P, F], f32, tag="wk", bufs=2)
                        for i, kt in enumerate(kts):
                            nc.tensor.matmul(
                                pf[:], attnT[kt][:, nt * P:(nt + 1) * P],
                                T["Wo"][kt][:, ds],
                                start=(i == 0), stop=(i == len(kts) - 1))
                            yield
                        p = partials[nt * 2 + dh]
                        nc.vector.tensor_add(
                            p[:], pf[:], bo_sb[:, ds] if first else p[:])
                        yield

            def out_gen():
                # chunks scheduled so each only reads finished attnT tiles:
                # (0,1)->dt2-3, (2,3)->dt4-5, (4,5)->dt6+7; kt 6 and 7 are
                # folded into the tail chains
                for kts, first in (((0, 1), True), ((2, 3), False),
                                   ((4, 5), False)):
                    for _ in out_chunk(kts, first):
                        yield

            def tail_gen(nts):
                # osb[nt,dh] = attnT[6:8]^T @ Wo[6:8] + partial -> DRAM
                for nt in nts:
                    for dh in range(2):
                        ds = slice(dh * F, (dh + 1) * F)
                        pf = ps.tile([P, F], f32, tag="wk", bufs=2)
                        for i, kt in enumerate((6, 7)):
                            nc.tensor.matmul(
                                pf[:], attnT[kt][:, nt * P:(nt + 1) * P],
                                T["Wo"][kt][:, ds],
                                start=(i == 0), stop=(i == 1))
                            yield
                        osb = stage.tile([P, F], bf16, tag="osb", bufs=4)
                        nc.vector.tensor_add(osb[:], pf[:],
                                             partials[nt * 2 + dh][:])
                        dq[(nt * 2 + dh) % 3].dma_start(
                            out=out_e[nt * P:(nt + 1) * P, ds], in_=osb[:])
                        yield

            # ---- S-pair + exp issue (attention front half) ----
            PTE_BUFS = 15
            pmap = {}

            def s_exp(dt, nh, mt):
                qt = qks[dt]["q"]
                ktt = qks[dt]["k"]
                ns = slice(nh * F, (nh + 1) * F)
                m0 = mt * P
                spair = ps.tile([P, 2 * F], f32, tag="spair", bufs=2)
                nc.tensor.matmul(spair[:, 0:F], ktt[0:HD, m0:m0 + P],
                                 qt[0:HD, ns], start=True, stop=True)
                nc.tensor.matmul(spair[:, F:2 * F], ktt[HD:P, m0:m0 + P],
                                 qt[HD:P, ns], start=True, stop=True)
                pte = stage.tile([P, 2 * F], bf16, tag="pte",
                                 bufs=PTE_BUFS)
                nc.scalar.activation(pte[:], spair[:], Exp)
                pmap[(dt, nh, mt)] = pte

            # ---- v projection with dt=0 S/exp pre-issue woven in ----
            VW = H * (HD + 1)  # 1040
            v_sb = [wpool.tile([P, VW], bf16, tag=f"v_{t}",
                               name=f"v_{t}") for t in range(8)]
            g0 = proj_gen(0)
            for _ in g0:
                pass

            sched0 = [(0, nh, mt) for nh in range(2) for mt in range(8)]
            s0 = 0
            for nt in range(8):
                vr = v_sb[nt][:].rearrange("p (h c) -> p h c", c=HD + 1)
                for dh in range(2):
                    ds = slice(dh * F, (dh + 1) * F)
                    pv = ps.tile([P, F], f32, tag="wk", bufs=2)
                    for kt in range(8):
                        nc.tensor.matmul(
                            pv[:], T["x"][kt][:, nt * P:(nt + 1) * P],
                            T["Wv"][kt][:, ds], start=(kt == 0),
                            stop=(kt == 7))
                    pvr = pv[:].rearrange("p (h c) -> p h c", c=HD)
                    nc.vector.tensor_copy(vr[:, dh * 8:(dh + 1) * 8, 0:HD],
                                          pvr[:])
                    if s0 < 12 and nt >= 1:
                        s_exp(*sched0[s0])
                        s0 += 1
                nc.vector.memset(vr[:, :, HD:HD + 1], 1.0)

            # ---- attention + weave ----
            def norm(dt, nh, po0, po1):
                ns = slice(nh * F, (nh + 1) * F)
                for hi, po in ((0, po0), (1, po1)):
                    # one copy frees the po PSUM bank; rest runs from SBUF
                    oah = stage.tile([HD + 1, F], f32, tag="oah", bufs=4)
                    nc.vector.tensor_copy(oah[:], po[:])
                    dn0 = stage.tile([1, F], f32, tag="dn0", bufs=3)
                    nc.sync.dma_start(out=dn0[:], in_=oah[HD:HD + 1, :])
                    rec32 = stage.tile([1, F], f32, tag="rec32", bufs=3)
                    nc.vector.reciprocal_approx_fast(rec32[:], dn0[:])
                    pbs = stage.tile([HD, F], f32, tag="pbs", bufs=3)
                    nc.gpsimd.partition_broadcast(pbs[:], rec32[0:1, :])
                    if hi == 0:
                        nc.vector.tensor_mul(attnT[dt][0:HD, ns],
                                             oah[0:HD, :], pbs[:])
                    else:
                        ast = stage.tile([HD, F], bf16, tag="ast", bufs=3)
                        nc.vector.tensor_mul(ast[:], oah[0:HD, :],
                                             pbs[:])
                        nc.sync.dma_start(out=attnT[dt][HD:P, ns],
                                          in_=ast[:])

            # Flat software pipeline over all 128 (dt, nh, mt) steps:
            # S(j) issues one step ahead of PV(j-1) so ACT runs exp
            # back-to-back across block boundaries.
            steps = [(dt, nh, mt) for dt in range(8) for nh in range(2)
                     for mt in range(8)]
            gens = {dt: proj_gen(dt + 1) for dt in range(7)}
            og = out_gen()

            # Per-dt weave: one full proj psum-group (9 yields), then out
            # chains -- strict alternation keeps the 2-buffer "wk" psum
            # pool from thrashing between concurrent accumulations.
            tg0 = None

            def dt_weave(dt):
                pg = gens.get(dt)
                on = 24 if dt >= 2 else 0
                if pg is None:
                    plan = ["o"] * 24 + ["t"] * 24
                else:
                    plan = []
                    for q in range(4):
                        plan += ["p"] * 9 + ["o"] * (on // 4)
                for c in plan:
                    if c == "p":
                        next(pg, None)
                    elif c == "o":
                        next(og, None)
                    else:
                        next(tg0, None)
                    yield
                if pg is not None:
                    for _ in pg:
                        pass

            def quotas(n):
                return [n * (s + 1) // 16 - n * s // 16 for s in range(16)]

            WQ = {dt: quotas(36 if dt < 2 else 60) for dt in range(7)}
            WQ[7] = [4, 4, 4, 4, 4, 4, 0, 0, 3, 3, 3, 3, 3, 3, 3, 3]
            wgens = {dt: dt_weave(dt) for dt in range(8)}
            # PV lags S by 2 steps so the exp latency (plus semaphore
            # jitter) never stalls the PE queue.
            LAG = 3
            pos = {}
            for j in range(128 + LAG):
                if 1 <= j < 128 and steps[j][1:] == (0, 0) and steps[j][0]:
                    # dt boundary: finish the next dt's q/k projection
                    # (python-level: binds qks[dt]) before issuing its S
                    for _ in wgens[steps[j][0] - 1]:
                        pass
                if j < 128:
                    dt, nh, mt = steps[j]
                    if mt == 0:
                        pos[(dt, nh)] = (
                            ps.tile([HD + 1, F], f32, tag="po", bufs=2,
                                    name=f"po0_{dt}_{nh}"),
                            ps.tile([HD + 1, F], f32, tag="po", bufs=2,
                                    name=f"po1_{dt}_{nh}"))
                    if not (dt == 0 and (nh * 8 + mt) < 12):
                        s_exp(dt, nh, mt)
                if j >= LAG:
                    dt, nh, mt = steps[j - LAG]
                    for _ in range(WQ[dt][nh * 8 + mt]):
                        next(wgens[dt], None)
                    h0 = 2 * dt
                    po0, po1 = pos[(dt, nh)]
                    pte = pmap.pop((dt, nh, mt))
                    nc.tensor.matmul(
                        po0[:], v_sb[mt][:, h0 * (HD + 1):
                                         (h0 + 1) * (HD + 1)],
                        pte[:, 0:F], start=(mt == 0), stop=(mt == 7))
                    nc.tensor.matmul(
                        po1[:], v_sb[mt][:, (h0 + 1) * (HD + 1):
                                         (h0 + 2) * (HD + 1)],
                        pte[:, F:2 * F], start=(mt == 0), stop=(mt == 7))
                    if mt == 7:
                        norm(dt, nh, po0, po1)
                        if (dt, nh) == (7, 0):
                            tg0 = tail_gen(range(4))

            # ---- output tail: remaining token tiles ----
            for _ in og:
                pass
            if tg0 is not None:
                for _ in tg0:
                    pass
            for _ in tail_gen(range(4, 8)):
                pass
    nc.compile()
    return nc


def _get_nc():
    if "nc" not in _CACHE:
        _CACHE["nc"] = _build()
    return _CACHE["nc"]


def _prep_shared(inputs):
    f = lambda a: np.asarray(a, np.float32)
    W = {}
    W["q"] = (f(inputs["Wq"]) + SCALING * (f(inputs["Bq"]) @ f(inputs["Aq"]))) * SCALE
    W["k"] = f(inputs["Wk"]) + SCALING * (f(inputs["Bk"]) @ f(inputs["Ak"]))
    W["v"] = f(inputs["Wv"]) + SCALING * (f(inputs["Bv"]) @ f(inputs["Av"]))
    W["o"] = f(inputs["Wo"]) + SCALING * (f(inputs["Bo"]) @ f(inputs["Ao"]))
    shared = {}
    for k, nm in (("q", "WqT"), ("k", "WkT"), ("v", "WvT"), ("o", "WoT")):
        shared[nm] = np.ascontiguousarray(W[k].T.astype(BF16))
    bo = f(inputs["bo"]).reshape(1, D)
    shared["boR"] = np.ascontiguousarray(
        np.broadcast_to(bo, (P, D)).astype(BF16))
    return shared


def kernel(**inputs):
    from concourse import bass_utils

    nc = _get_nc()
    shared = _prep_shared(inputs)
    x = np.asarray(inputs["x"], np.float32)
    in_maps = []
    for i in range(NCORES):
        m = dict(shared)
        m["xT"] = np.ascontiguousarray(x[i].T.astype(BF16))
        in_maps.append(m)
    res = bass_utils.run_bass_kernel_spmd(nc, in_maps,
                                          core_ids=list(range(NCORES)))
    return np.stack([np.asarray(res.results[i]["out"]).astype(np.float32)
                     for i in range(NCORES)], axis=0)


# revision 42
# speedup vs baseline: 1.2075x; 1.2075x over previous
"""LoRA multi-head attention on 8 TRN2 NeuronCores.

Sharding: data-parallel over batch (B=8 -> 1 batch element per core),
weights replicated, no collectives.

Host side: LoRA is folded into the dense weights exactly
(W' = W + (alpha/r) * B @ A), the attention scale 1/8 is folded into
Wq (power of two => lossless), and bo is pre-replicated across
partitions. The device then runs a plain dense MHA in bf16 with fp32
PSUM accumulation.

Device pipeline per core:
  qT/kT per dout-tile dt: (WT.T @ xT); v natural per token-tile with a
  ones column ([v_h | 1]) so PV also yields softmax denominators.
  Attention per dt (2 heads), per query-half nh, per key-tile mt:
    S^T pair via 64-row PE tiling (both heads concurrent) -> exp on
    ACT (psum -> bf16 sbuf) -> PV (M=65).  The next dout-tile's q/k
    projection matmuls are woven between S and PV so the PE never
    waits on the ACT exp latency.
  Normalization: reciprocal of the denom row (bf16) broadcast via a
  K=1 bf16 matmul; DVE multiply writes attnT directly (head 0) or via
  a staging tile + SBUF DMA (head 1).
  Output projection: kt=0..6 partial sums woven into the dt=7
  attention loop (+bo), tail adds only the kt=7 term.
"""

import sys

if "/opt/trn_rl_repo" not in sys.path:
    sys.path.insert(0, "/opt/trn_rl_repo")

import numpy as np
import ml_dtypes

BF16 = ml_dtypes.bfloat16

N = 1024  # tokens
D = 1024  # model dim
H = 16    # heads
HD = 64   # head dim
R = 16    # lora rank
P = 128   # partitions
F = 512   # psum free-dim tile
NCORES = 8
SCALING = 1.0 / 16.0  # lora alpha/rank
SCALE = HD ** -0.5

_CACHE = {}


def _build():
    import concourse.bacc as bacc
    import concourse.mybir as mybir
    import concourse.tile as tile

    f32 = mybir.dt.float32
    bf16 = mybir.dt.bfloat16
    Exp = mybir.ActivationFunctionType.Exp

    nc = bacc.Bacc("TRN2", target_bir_lowering=False, debug=False)

    xT_e = nc.declare_dram_parameter("xT", [D, N], bf16, isOutput=False)
    wT_e = {
        nm: nc.declare_dram_parameter(nm, [D, D], bf16, isOutput=False)
        for nm in ("WqT", "WkT", "WvT", "WoT")
    }
    bo_e = nc.declare_dram_parameter("boR", [P, D], bf16, isOutput=False)
    out_e = nc.declare_dram_parameter("out", [N, D], bf16, isOutput=True)

    with tile.TileContext(nc) as tc:
        with (
            tc.tile_pool(name="wpool", bufs=1) as wpool,
            tc.tile_pool(name="stage", bufs=2) as stage,
            tc.tile_pool(name="ps", bufs=1, space="PSUM") as ps,
        ):
            # ---- DMA loads: x first (5 queues), then weights ----
            dq = [nc.sync, nc.scalar, nc.gpsimd]
            qi = 0
            T = {}

            def load_big(nm, ext, qs3=False):
                nonlocal qi
                T[nm] = []
                for t in range(8):
                    tt = wpool.tile([P, D], bf16, tag=f"T_{nm}_{t}",
                                    name=f"T_{nm}_{t}")
                    eng = dq[qi % 3] if qs3 else [nc.sync, nc.gpsimd][qi % 2]
                    eng.dma_start(out=tt[:], in_=ext[t * P:(t + 1) * P, :])
                    qi += 1
                    T[nm].append(tt)

            # x spread over 3 queues (gates everything); Wq/Wk dt=0/1
            # columns sliced so dt=0 projections start at ~9us; Wv next
            # (v-projection); the rest as big region loads behind compute.
            # Steady-state queue roles: sync=ast/out, scalar=exp + dn0,
            # gpsimd=partition_broadcast.
            load_big("x", xT_e, qs3=True)
            wq2 = [nc.sync, nc.gpsimd]

            def load_qk_cols(dts):
                nonlocal qi
                c0, c1 = dts[0] * P, dts[-1] * P + P
                for nm in ("Wq", "Wk"):
                    if nm not in T:
                        T[nm] = [wpool.tile([P, D], bf16,
                                            tag=f"T_{nm}_{t}",
                                            name=f"T_{nm}_{t}")
                                 for t in range(8)]
                    for t in range(8):
                        wq2[qi % 2].dma_start(
                            out=T[nm][t][:, c0:c1],
                            in_=wT_e[nm + "T"][t * P:(t + 1) * P, c0:c1])
                        qi += 1

            load_qk_cols([0])
            load_big("Wv", wT_e["WvT"])
            load_qk_cols([1])
            load_qk_cols([2, 3, 4, 5, 6, 7])
            load_big("Wo", wT_e["WoT"])
            bo_sb = wpool.tile([P, D], bf16, tag="bo")
            nc.gpsimd.dma_start(out=bo_sb[:], in_=bo_e[:, :])

            ones1 = wpool.tile([1, HD], bf16, tag="ones1")
            nc.vector.memset(ones1[:], 1.0)
            warm = wpool.tile([P, F], bf16, tag="warm")
            nc.vector.memset(warm[:], 0.0)

            # ---- ACT exp-table preload + PE warm-up (no DMA deps) ----
            wexp = stage.tile([P, F], bf16, tag="wexp", bufs=1)
            wps = ps.tile([P, F], f32, tag="wk", bufs=2)
            nc.tensor.matmul(wps[:], warm[:, 0:P], warm[:], start=True,
                             stop=True)
            nc.scalar.activation(wexp[:], wps[:], Exp)
            for _ in range(23):
                nc.tensor.matmul(wps[:], warm[:, 0:P], warm[:],
                                 start=True, stop=True)

            # ---- q/k projection generator for dout tile dt ----
            qks = {}

            def proj_gen(dt):
                qk = {}
                for nm, wnm in (("q", "Wq"), ("k", "Wk")):
                    dst = wpool.tile([P, D], bf16, tag=f"{nm}T",
                                     bufs=3, name=f"{nm}T_{dt}")
                    qk[nm] = dst
                    for nh in range(2):
                        ns = slice(nh * F, (nh + 1) * F)
                        pq = ps.tile([P, F], f32, tag="wk", bufs=2)
                        for kt in range(8):
                            nc.tensor.matmul(
                                pq[:], T[wnm][kt][:, dt * P:(dt + 1) * P],
                                T["x"][kt][:, ns],
                                start=(kt == 0), stop=(kt == 7))
                            yield
                        nc.vector.tensor_copy(dst[:, ns], pq[:])
                        yield
                qks[dt] = qk

            # ---- output-projection partials (kt=0..6) for dt=7 weave ----
            attnT = [wpool.tile([P, D], bf16, tag=f"attnT_{t}",
                                name=f"attnT_{t}") for t in range(8)]
            partials = [wpool.tile([P, F], bf16, tag=f"part_{t}",
                                   name=f"part_{t}") for t in range(16)]

            def out_chunk(kts, first):
                # partial[nt,dh] (+)= sum_{kt in kts} attnT[kt]^T @ Wo[kt]
                for nt in range(8):
                    for dh in range(2):
                        ds = slice(dh * F, (dh + 1) * F)
                        pf = ps.tile([P, F], f32, tag="wk", bufs=2)
                        for i, kt in enumerate(kts):
                            nc.tensor.matmul(
                                pf[:], attnT[kt][:, nt * P:(nt + 1) * P],
                                T["Wo"][kt][:, ds],
                                start=(i == 0), stop=(i == len(kts) - 1))
                            yield
                        p = partials[nt * 2 + dh]
                        nc.vector.tensor_add(
                            p[:], pf[:], bo_sb[:, ds] if first else p[:])
                        yield

            def out_gen():
                # chunks scheduled so each only reads finished attnT tiles:
                # (0,1)->dt2-3, (2,3)->dt4-5, (4,5)->dt6+7; kt 6 and 7 are
                # folded into the tail chains
                for kts, first in (((0, 1), True), ((2, 3), False),
                                   ((4, 5), False)):
                    for _ in out_chunk(kts, first):
                        yield

            def tail_gen(nts):
                # osb[nt,dh] = attnT[6:8]^T @ Wo[6:8] + partial -> DRAM
                for nt in nts:
                    for dh in range(2):
                        ds = slice(dh * F, (dh + 1) * F)
                        pf = ps.tile([P, F], f32, tag="wk", bufs=2)
                        for i, kt in enumerate((6, 7)):
                            nc.tensor.matmul(
                                pf[:], attnT[kt][:, nt * P:(nt + 1) * P],
                                T["Wo"][kt][:, ds],
                                start=(i == 0), stop=(i == 1))
                            yield
                        osb = stage.tile([P, F], bf16, tag="osb", bufs=4)
                        nc.vector.tensor_add(osb[:], pf[:],
                                             partials[nt * 2 + dh][:])
                        dq[(nt * 2 + dh) % 3].dma_start(
                            out=out_e[nt * P:(nt + 1) * P, ds], in_=osb[:])
                        yield

            # ---- S-pair + exp issue (attention front half) ----
            PTE_BUFS = 15
            pmap = {}

            def s_exp(dt, nh, mt):
                qt = qks[dt]["q"]
                ktt = qks[dt]["k"]
                ns = slice(nh * F, (nh + 1) * F)
                m0 = mt * P
                spair = ps.tile([P, 2 * F], f32, tag="spair", bufs=2)
                nc.tensor.matmul(spair[:, 0:F], ktt[0:HD, m0:m0 + P],
                                 qt[0:HD, ns], start=True, stop=True)
                nc.tensor.matmul(spair[:, F:2 * F], ktt[HD:P, m0:m0 + P],
                                 qt[HD:P, ns], start=True, stop=True)
                pte = stage.tile([P, 2 * F], bf16, tag="pte",
                                 bufs=PTE_BUFS)
                nc.scalar.activation(pte[:], spair[:], Exp)
                pmap[(dt, nh, mt)] = pte

            # ---- v projection with dt=0 S/exp pre-issue woven in ----
            VW = H * (HD + 1)  # 1040
            v_sb = [wpool.tile([P, VW], bf16, tag=f"v_{t}",
                               name=f"v_{t}") for t in range(8)]
            g0 = proj_gen(0)
            for _ in g0:
                pass

            sched0 = [(0, nh, mt) for nh in range(2) for mt in range(8)]
            s0 = 0
            for nt in range(8):
                vr = v_sb[nt][:].rearrange("p (h c) -> p h c", c=HD + 1)
                for dh in range(2):
                    ds = slice(dh * F, (dh + 1) * F)
                    pv = ps.tile([P, F], f32, tag="wk", bufs=2)
                    for kt in range(8):
                        nc.tensor.matmul(
                            pv[:], T["x"][kt][:, nt * P:(nt + 1) * P],
                            T["Wv"][kt][:, ds], start=(kt == 0),
                            stop=(kt == 7))
                    pvr = pv[:].rearrange("p (h c) -> p h c", c=HD)
                    nc.vector.tensor_copy(vr[:, dh * 8:(dh + 1) * 8, 0:HD],
                                          pvr[:])
                    if s0 < 12 and nt >= 1:
                        s_exp(*sched0[s0])
                        s0 += 1
                nc.vector.memset(vr[:, :, HD:HD + 1], 1.0)

            # ---- attention + weave ----
            def norm(dt, nh, po0, po1):
                ns = slice(nh * F, (nh + 1) * F)
                for hi, po in ((0, po0), (1, po1)):
                    # one copy frees the po PSUM bank; rest runs from SBUF
                    oah = stage.tile([HD + 1, F], f32, tag="oah", bufs=4)
                    nc.vector.tensor_copy(oah[:], po[:])
                    dn0 = stage.tile([1, F], f32, tag="dn0", bufs=3)
                    nc.sync.dma_start(out=dn0[:], in_=oah[HD:HD + 1, :])
                    rec32 = stage.tile([1, F], f32, tag="rec32", bufs=3)
                    nc.vector.reciprocal_approx_fast(rec32[:], dn0[:])
                    pbs = stage.tile([HD, F], f32, tag="pbs", bufs=3)
                    nc.gpsimd.partition_broadcast(pbs[:], rec32[0:1, :])
                    if hi == 0:
                        nc.vector.tensor_mul(attnT[dt][0:HD, ns],
                                             oah[0:HD, :], pbs[:])
                    else:
                        ast = stage.tile([HD, F], bf16, tag="ast", bufs=3)
                        nc.vector.tensor_mul(ast[:], oah[0:HD, :],
                                             pbs[:])
                        nc.sync.dma_start(out=attnT[dt][HD:P, ns],
                                          in_=ast[:])

            # Flat software pipeline over all 128 (dt, nh, mt) steps:
            # S(j) issues one step ahead of PV(j-1) so ACT runs exp
            # back-to-back across block boundaries.
            steps = [(dt, nh, mt) for dt in range(8) for nh in range(2)
                     for mt in range(8)]
            gens = {dt: proj_gen(dt + 1) for dt in range(7)}
            og = out_gen()

            # Per-dt weave: one full proj psum-group (9 yields), then out
            # chains -- strict alternation keeps the 2-buffer "wk" psum
            # pool from thrashing between concurrent accumulations.
            tg0 = None

            def dt_weave(dt):
                pg = gens.get(dt)
                on = 24 if dt >= 2 else 0
                if pg is None:
                    plan = ["o"] * 24 + ["t"] * 24
                else:
                    plan = []
                    for q in range(4):
                        plan += ["p"] * 9 + ["o"] * (on // 4)
                for c in plan:
                    if c == "p":
                        next(pg, None)
                    elif c == "o":
                        next(og, None)
                    else:
                        next(tg0, None)
                    yield
                if pg is not None:
                    for _ in pg:
                        pass

            def quotas(n):
                return [n * (s + 1) // 16 - n * s // 16 for s in range(16)]

            WQ = {dt: quotas(36 if dt < 2 else 60) for dt in range(7)}
            WQ[7] = [4, 4, 4, 4, 4, 4, 0, 0, 3, 3, 3, 3, 3, 3, 3, 3]
            wgens = {dt: dt_weave(dt) for dt in range(8)}
            # PV lags S by 2 steps so the exp latency (plus semaphore
            # jitter) never stalls the PE queue.
            LAG = 3
            pos = {}
            for j in range(128 + LAG):
                if 1 <= j < 128 and steps[j][1:] == (0, 0) and steps[j][0]:
                    # dt boundary: finish the next dt's q/k projection
                    # (python-level: binds qks[dt]) before issuing its S
                    for _ in wgens[steps[j][0] - 1]:
                        pass
                if j < 128:
                    dt, nh, mt = steps[j]
                    if mt == 0:
                        pos[(dt, nh)] = (
                            ps.tile([HD + 1, F], f32, tag="po", bufs=2,
                                    name=f"po0_{dt}_{nh}"),
                            ps.tile([HD + 1, F], f32, tag="po", bufs=2,
                                    name=f"po1_{dt}_{nh}"))
                    if not (dt == 0 and (nh * 8 + mt) < 12):
                        s_exp(dt, nh, mt)
                if j >= LAG:
                    dt, nh, mt = steps[j - LAG]
                    for _ in range(WQ[dt][nh * 8 + mt]):
                        next(wgens[dt], None)
                    h0 = 2 * dt
                    po0, po1 = pos[(dt, nh)]
                    pte = pmap.pop((dt, nh, mt))
                    nc.tensor.matmul(
                        po0[:], v_sb[mt][:, h0 * (HD + 1):
                                         (h0 + 1) * (HD + 1)],
                        pte[:, 0:F], start=(mt == 0), stop=(mt == 7))
                    nc.tensor.matmul(
                        po1[:], v_sb[mt][:, (h0 + 1) * (HD + 1):
                                         (h0 + 2) * (HD + 1)],
                        pte[:, F:2 * F], start=(mt == 0), stop=(mt == 7))
                    if mt == 7:
                        norm(dt, nh, po0, po1)
                        if (dt, nh) == (7, 0):
                            tg0 = tail_gen(range(4))

            # ---- output tail: remaining token tiles ----
            for _ in og:
                pass
            if tg0 is not None:
                for _ in tg0:
                    pass
            for _ in tail_gen(range(4, 8)):
                pass
    nc.compile()
    return nc


def _get_nc():
    if "nc" not in _CACHE:
        _CACHE["nc"] = _build()
    return _CACHE["nc"]


def _prep_shared(inputs):
    f = lambda a: np.asarray(a, np.float32)
    W = {}
    W["q"] = (f(inputs["Wq"]) + SCALING * (f(inputs["Bq"]) @ f(inputs["Aq"]))) * SCALE
    W["k"] = f(inputs["Wk"]) + SCALING * (f(inputs["Bk"]) @ f(inputs["Ak"]))
    W["v"] = f(inputs["Wv"]) + SCALING * (f(inputs["Bv"]) @ f(inputs["Av"]))
    W["o"] = f(inputs["Wo"]) + SCALING * (f(inputs["Bo"]) @ f(inputs["Ao"]))
    shared = {}
    for k, nm in (("q", "WqT"), ("k", "WkT"), ("v", "WvT"), ("o", "WoT")):
        shared[nm] = np.ascontiguousarray(W[k].T.astype(BF16))
    bo = f(inputs["bo"]).reshape(1, D)
    shared["boR"] = np.ascontiguousarray(
        np.broadcast_to(bo, (P, D)).astype(BF16))
    return shared


def kernel(**inputs):
    from concourse import bass_utils

    nc = _get_nc()
    shared = _prep_shared(inputs)
    x = np.asarray(inputs["x"], np.float32)
    in_maps = []
    for i in range(NCORES):
        m = dict(shared)
        m["xT"] = np.ascontiguousarray(x[i].T.astype(BF16))
        in_maps.append(m)
    res = bass_utils.run_bass_kernel_spmd(nc, in_maps,
                                          core_ids=list(range(NCORES)))
    return np.stack([np.asarray(res.results[i]["out"]).astype(np.float32)
                     for i in range(NCORES)], axis=0)
